# revision 2
# baseline (speedup 1.0000x reference)
"""Trainium2 Bass kernel for nn_Attention_1013612281902 (v2: bf16 + pair collectives).

Math (per batch b, head h), restructured to avoid Q/K/V materialization:
    G = emb.T @ emb_all                     [C,KV]  (shared across heads)
    scores_h = Wq[h] @ G @ Wk[h].T          [C,KV]  (unscaled; instance-norm is
                                                     scale-invariant, eps adjusted)
    probs_h  = softmax(instnorm(scores_h))
    Pv_h     = probs_h @ Wv[h]              [C,KV]
    y        = emb_all @ (mean_h Pv_h).T @ Wo.T     [S,C]

Sharding: 8 cores = (4 batches) x (2 pair-members). Within a batch pair:
  - member g owns 4 of the 8 KV k-tiles and computes G only over those columns;
    a pair AllGather assembles the full G on both cores in canonical order.
  - member g computes heads {2g, 2g+1}; per-head Pv partials are pair
    ReduceScattered (sum over members, shard over j2-tiles) so each member
    gets the fully-summed Pbar rows for its own k-tiles.
  - member g computes the partial y = emb_all[:, own] @ Z[own] over ALL rows;
    the host adds the two partials per batch (y is linear in Z's row blocks).
The per-core "own tiles first" view is produced host-side by permuting
emb_all's column blocks per core (eaL), so one fixed SPMD program serves both
pair members; collective rank order restores canonical tile order.

Everything on the PE runs bf16 (f32 PSUM accumulation, f32 stats/softmax
scalars); measured end-to-end error ~6e-3 vs the 2e-2 gate.
"""

import sys

DEBUG = False

if "/opt/trn_rl_repo" not in sys.path:
    sys.path.insert(0, "/opt/trn_rl_repo")

from contextlib import ExitStack

import numpy as np

import concourse.bacc as bacc
import concourse.mybir as mybir
import concourse.tile as tile
from concourse.bass_utils import run_bass_kernel_spmd
from concourse.masks import make_identity
from concourse.tile_rust import add_dep_helper

B, S, C, KV, H = 4, 4096, 512, 960, 4
KVP = 1024              # padded KV
EPS = 1e-5
F32 = mybir.dt.float32
BF16 = mybir.dt.bfloat16

ST = S // 128           # 32 s-tiles
CT = C // 128           # 4 c-tiles
KT = KVP // 128         # 8 k-tiles (padded; tile 7 holds 64 real rows)
OWN = 4                 # owned k-tiles per pair member

PAIRS = [[0, 1], [2, 3], [4, 5], [6, 7]]


def _jp(t):
    """valid j rows in tile t of the unpadded 960 axis"""
    return min(128, KV - t * 128)


def _build_program():
    nc = bacc.Bacc("TRN2", target_bir_lowering=False, debug=False, num_devices=8)

    emb_d = nc.dram_tensor("emb", [S, C], BF16, kind="ExternalInput")
    ea_d = nc.dram_tensor("ea", [S, KVP], BF16, kind="ExternalInput")
    wqt_d = nc.dram_tensor("wqt", [2, C, C], BF16, kind="ExternalInput")
    wkt_d = nc.dram_tensor("wkt", [2, KV, KV], BF16, kind="ExternalInput")
    wv_d = nc.dram_tensor("wv", [2, KV, KV], BF16, kind="ExternalInput")
    wot_d = nc.dram_tensor("wot", [C, C], BF16, kind="ExternalInput")
    y_d = nc.dram_tensor("y", [S, C], BF16, kind="ExternalOutput")
    if DEBUG:
        dbg_g = nc.dram_tensor("dbg_g", [128, CT, KVP], BF16, kind="ExternalOutput")
        dbg_st = nc.dram_tensor("dbg_st", [128, 64], F32, kind="ExternalOutput")
        dbg_pr = nc.dram_tensor("dbg_pr", [128, KT, C], BF16, kind="ExternalOutput")
        dbg_sg = nc.dram_tensor("dbg_sg", [128, KT, C], BF16, kind="ExternalOutput")
        dbg_rs = nc.dram_tensor("dbg_rs", [128, OWN, C], BF16, kind="ExternalOutput")
        dbg_eat = nc.dram_tensor("dbg_eat", [128, OWN, 512], BF16, kind="ExternalOutput")
        dbg_z = nc.dram_tensor("dbg_z", [128, OWN, C], BF16, kind="ExternalOutput")
        dbg_rs1 = nc.dram_tensor("dbg_rs1", [128, OWN, C], BF16, kind="ExternalOutput")
        dbg_pbt = nc.dram_tensor("dbg_pbt", [2, 128, CT, C], BF16, kind="ExternalOutput")

    with tile.TileContext(nc) as tc, ExitStack() as ectx:
        ec = ectx.enter_context
        const = ec(tc.tile_pool(name="const", bufs=1))
        gp = ec(tc.tile_pool(name="gp", bufs=1))
        eatp = ec(tc.tile_pool(name="eatp", bufs=1))
        wqp = ec(tc.tile_pool(name="wqp", bufs=1))
        wkp = ec(tc.tile_pool(name="wkp", bufs=1))
        wvp = ec(tc.tile_pool(name="wvp", bufs=1))
        wop = ec(tc.tile_pool(name="wop", bufs=1))
        embp = ec(tc.tile_pool(name="embp", bufs=6))
        eap = ec(tc.tile_pool(name="eap", bufs=5))
        bigp = ec(tc.tile_pool(name="bigp", bufs=1))   # A tiles (2 heads serialized)
        scp = ec(tc.tile_pool(name="scp", bufs=1))     # scoresT -> probsT
        stgp = ec(tc.tile_pool(name="stgp", bufs=2))   # Pv staging per head
        outp = ec(tc.tile_pool(name="outp", bufs=4))
        srp = ec(tc.tile_pool(name="srp", bufs=2))     # [128,512] scratch
        stp = ec(tc.tile_pool(name="stp", bufs=4))     # small stats tiles
        dramp = ec(tc.tile_pool(name="dramp", bufs=1, space="DRAM"))

        identf = const.tile([128, 128], F32)
        make_identity(nc, identf[:])
        ident = const.tile([128, 128], BF16)
        nc.vector.tensor_copy(out=ident[:], in_=identf[:])
        onesf = const.tile([128, 128], F32)
        nc.vector.memset(onesf[:], 1.0)
        ones = const.tile([128, 128], BF16)
        nc.vector.tensor_copy(out=ones[:], in_=onesf[:])
        # scores are left unscaled (instance-norm is scale-invariant), so the
        # reference's eps applies to var/KV: use KV*eps against unscaled var.
        eps_t = const.tile([128, 1], F32)
        nc.vector.memset(eps_t[:], EPS * KV)
        zsrc = const.tile([128, KV], BF16)
        nc.vector.memset(zsrc[:], 0.0)
        warm = const.tile([128, 1], F32)
        nc.vector.memset(warm[:], 1.0)

        def prewarm(func, nm):
            wsink = stp.tile([128, 1], F32, tag="wsink", name=nm)
            nc.scalar.activation(out=wsink[:], in_=warm[:], func=func)

        # ---- phase 1: G_own = emb.T @ eaL[:, own]  +  eaT transposes --------
        eat_sb = eatp.tile([128, OWN, S], BF16)  # eaL[:, own].T resident
        g_sb = gp.tile([128, CT, KVP], BF16)

        p1_pool = tc.tile_pool(name="p1ps", bufs=1, space="PSUM")
        ps = p1_pool.__enter__()
        g_ps = [
            ps.tile([128, 512], F32, tag=f"g{i}", name=f"g_ps{i}") for i in range(CT)
        ]
        for st in range(ST):
            et = embp.tile([128, C], BF16, tag="emb", name=f"et{st}")
            nc.sync.dma_start(out=et[:], in_=emb_d.ap()[st * 128 : (st + 1) * 128, :])
            at = eap.tile([128, KVP], BF16, tag="ea", name=f"at{st}")
            nc.sync.dma_start(out=at[:], in_=ea_d.ap()[st * 128 : (st + 1) * 128, :])
            for ct in range(CT):
                nc.tensor.matmul(
                    g_ps[ct][:],
                    et[:, ct * 128 : (ct + 1) * 128],
                    at[:, 0:512],
                    start=(st == 0),
                    stop=(st == ST - 1),
                )
            for t in range(OWN):
                ptc = ps.tile([128, 128], BF16, tag="ptc", bufs=3, name=f"p1t{st}{t}")
                nc.tensor.transpose(
                    ptc[:], at[:, t * 128 : (t + 1) * 128], ident[:]
                )
                dst = eat_sb[:, t, st * 128 : (st + 1) * 128]
                if t % 2 == 0:
                    nc.vector.tensor_copy(out=dst, in_=ptc[:])
                else:
                    nc.scalar.copy(out=dst, in_=ptc[:])

        # ---- weights (consumption order; stream during AG window) ----------
        wqt_sb = []
        wkt_sb = []
        wv_sb = []
        for h in range(2):
            wq_t = wqp.tile([128, CT, C], BF16, tag="wq", name=f"wq{h}")
            nc.sync.dma_start(
                out=wq_t[:],
                in_=wqt_d.ap()[h].rearrange("(t p) d -> p t d", p=128),
            )
            wqt_sb.append(wq_t)
            wk_t = wkp.tile([128, KT, KV], BF16, tag="wk", name=f"wk{h}")
            wv_t = wvp.tile([128, KT, KV], BF16, tag="wv", name=f"wv{h}")
            for kt in range(KT):
                kp = _jp(kt)
                nc.sync.dma_start(
                    out=wk_t[:kp, kt, :],
                    in_=wkt_d.ap()[h, kt * 128 : kt * 128 + kp, :],
                )
            for kt in range(KT):
                kp = _jp(kt)
                nc.sync.dma_start(
                    out=wv_t[:kp, kt, :],
                    in_=wv_d.ap()[h, kt * 128 : kt * 128 + kp, :],
                )
            nc.vector.tensor_copy(out=wk_t[64:128, KT - 1, :], in_=zsrc[64:128, :])
            nc.vector.tensor_copy(out=wv_t[64:128, KT - 1, :], in_=zsrc[64:128, :])
            wkt_sb.append(wk_t)
            wv_sb.append(wv_t)
        wot_sb = wop.tile([128, CT, C], BF16)
        nc.sync.dma_start(
            out=wot_sb[:], in_=wot_d.ap().rearrange("(t p) d -> p t d", p=128)
        )

        # ---- G AllGather over the pair --------------------------------------
        gtmp = srp.tile([128, CT, 512], BF16, tag="sr", name="gtmp")
        for ct in range(CT):
            if ct % 2 == 0:
                nc.vector.tensor_copy(out=gtmp[:, ct, :], in_=g_ps[ct][:])
            else:
                nc.scalar.copy(out=gtmp[:, ct, :], in_=g_ps[ct][:])
        g_bin = dramp.tile([512, 512], BF16, name="g_bin")
        for ct in range(CT):
            nc.gpsimd.dma_start(
                g_bin[ct * 128 : (ct + 1) * 128, :], gtmp[:, ct, :]
            )
        g_bout = dramp.tile([2, 512, 512], BF16, name="g_bout")
        nc.gpsimd.collective_compute(
            "AllGather",
            mybir.AluOpType.bypass,
            replica_groups=PAIRS,
            ins=[g_bin[:].opt()],
            outs=[g_bout[:].opt()],
        )
        for blk in range(2):
            for ct in range(CT):
                nc.sync.dma_start(
                    out=g_sb[:, ct, blk * 512 : (blk + 1) * 512],
                    in_=g_bout[blk, ct * 128 : (ct + 1) * 128, :],
                )
        p1_pool.__exit__(None, None, None)

        # ---- phase 2: per-head scores -> instancenorm -> softmax -> Pv ------
        ph2_pool = tc.tile_pool(name="ph2ps", bufs=1, space="PSUM")
        ps = ph2_pool.__enter__()
        hs = [{}, {}]
        rs_out = []

        def emit_A(h):
            d = hs[h]
            d["a_sb"] = a_sb = bigp.tile(
                [128, KT, C], BF16, tag="big", name=f"a_sb{h}"
            )
            for kt in range(KT):
                pa = ps.tile([128, C], F32, tag="psa", bufs=2, name=f"pa{h}{kt}")
                for ct in range(CT):
                    nc.tensor.matmul(
                        pa[:],
                        g_sb[:, ct, kt * 128 : (kt + 1) * 128],
                        wqt_sb[h][:, ct, :],
                        start=(ct == 0),
                        stop=(ct == CT - 1),
                    )
                nc.vector.tensor_copy(out=a_sb[:, kt, :], in_=pa[:])

        def emit_scoresT(h):
            # scoresT[j, d] = sum_k WkT[k,j] A[k,d]; per-jt stats partials run
            # inline right behind each group.
            d = hs[h]
            a_sb = d["a_sb"]
            d["sc_sb"] = sc_sb = scp.tile(
                [128, KT, C], BF16, tag="sc", name=f"sc_sb{h}"
            )
            d["p_sb"] = p_sb = stp.tile([128, 16], F32, tag="p16", name=f"p_sb{h}")
            nc.vector.memset(p_sb[:], 0.0)
            prev_stop = None
            for jt in range(KT):
                jp = _jp(jt)
                pss = ps.tile([128, C], F32, tag="pw", bufs=4, name=f"pss{h}{jt}")
                for kt in range(KT):
                    mm = nc.tensor.matmul(
                        pss[:jp, :],
                        wkt_sb[h][:, kt, jt * 128 : jt * 128 + jp],
                        a_sb[:, kt, :],
                        start=(kt == 0),
                        stop=(kt == KT - 1),
                    )
                    # Keep the PE stream jt-group-major so stats can chase.
                    if kt == 0 and prev_stop is not None:
                        add_dep_helper(
                            mm.ins, prev_stop.ins, sync=False, reason="jt order"
                        )
                    if kt == KT - 1:
                        prev_stop = mm
                nc.scalar.copy(out=sc_sb[:jp, jt, :], in_=pss[:jp, :])
                nc.vector.reduce_sum(
                    out=p_sb[:jp, jt : jt + 1],
                    in_=pss[:jp, :],
                    axis=mybir.AxisListType.X,
                )
                nc.scalar.activation(
                    out=pss[:jp, :],
                    in_=pss[:jp, :],
                    func=mybir.ActivationFunctionType.Square,
                    accum_out=p_sb[:jp, 8 + jt : 9 + jt],
                )
            nc.vector.tensor_copy(out=sc_sb[64:128, KT - 1, :], in_=zsrc[64:128, :C])

        def emit_softmax_pv(h):
            d = hs[h]
            sc_sb = d["sc_sb"]
            p_sb = d["p_sb"]
            # cross-partition reduce + broadcast of the plane stats.
            p_r = stp.tile([128, 16], BF16, tag="p16r", name=f"p_r{h}")
            nc.vector.tensor_copy(out=p_r[:], in_=p_sb[:])
            pst = ps.tile([128, 16], F32, tag="one", bufs=2, name=f"pst{h}")
            nc.tensor.matmul(pst[:], ones[:], p_r[:], start=True, stop=True)
            n_inv = 1.0 / float(C * KV)
            sq2 = stp.tile([128, 2], F32, tag="sq2", name=f"sq2{h}")
            nc.vector.reduce_sum(
                out=sq2[:],
                in_=pst[:].rearrange("p (a b) -> p a b", a=2),
                axis=mybir.AxisListType.X,
            )
            # mean_neg = -sum/N; em2 = sumsq/N
            mean_neg = stp.tile([128, 1], F32, tag="mean", name=f"mean{h}")
            nc.vector.tensor_scalar(
                out=mean_neg[:], in0=sq2[:, 0:1], scalar1=-n_inv, scalar2=None,
                op0=mybir.AluOpType.mult,
            )
            em2 = stp.tile([128, 1], F32, tag="em2", name=f"em2{h}")
            nc.vector.tensor_scalar(
                out=em2[:], in0=sq2[:, 1:2], scalar1=n_inv, scalar2=None,
                op0=mybir.AluOpType.mult,
            )
            m2 = stp.tile([128, 1], F32, tag="m2", name=f"m2{h}")
            nc.vector.tensor_mul(out=m2[:], in0=mean_neg[:], in1=mean_neg[:])
            var_t = stp.tile([128, 1], F32, tag="var", name=f"var{h}")
            nc.vector.tensor_sub(out=var_t[:], in0=em2[:], in1=m2[:])
            std_t = stp.tile([128, 1], F32, tag="std", name=f"std{h}")
            nc.scalar.activation(
                out=std_t[:],
                in_=var_t[:],
                func=mybir.ActivationFunctionType.Sqrt,
                bias=eps_t[:],
            )
            # Swap the ACT table back to Exp while the DVE finishes the chain.
            prewarm(mybir.ActivationFunctionType.Exp, f"wex{h}")
            rstd_t = stp.tile([128, 1], F32, tag="rstd", name=f"rstd{h}")
            nc.vector.reciprocal(out=rstd_t[:], in_=std_t[:])
            negmr = stp.tile([128, 1], F32, tag="negmr", name=f"negmr{h}")
            nc.vector.tensor_mul(out=negmr[:], in0=mean_neg[:], in1=rstd_t[:])
            if DEBUG and h == 0:
                dstt = stp.tile([128, 64], F32, tag="dstt", name="dstt")
                nc.vector.memset(dstt[:], 0.0)
                nc.vector.tensor_copy(out=dstt[:, 0:16], in_=p_sb[:])
                nc.vector.tensor_copy(out=dstt[:, 16:17], in_=mean_neg[:])
                nc.vector.tensor_copy(out=dstt[:, 17:18], in_=var_t[:])
                nc.vector.tensor_copy(out=dstt[:, 18:19], in_=rstd_t[:])
                nc.vector.tensor_copy(out=dstt[:, 19:20], in_=negmr[:])
                nc.vector.tensor_copy(out=dstt[:, 20:36], in_=pst[:])
                nc.sync.dma_start(out=dbg_st.ap()[:], in_=dstt[:])

            # softmax + Pv fused: Pv matmuls consume raw exp tiles as they
            # are produced; 1/denom and the 0.25 head-mean factor fold into
            # the staging copy-out.
            psd = ps.tile([128, C], F32, tag="one", bufs=2, name=f"psd{h}")
            pp_w1 = [
                ps.tile([128, C], F32, tag="pw", bufs=4, name=f"pp{h}w1_{kt}")
                for kt in range(4)
            ]
            for jt in range(KT):
                jp = _jp(jt)
                nc.scalar.activation(
                    out=sc_sb[:jp, jt, :],
                    in_=sc_sb[:jp, jt, :],
                    func=mybir.ActivationFunctionType.Exp,
                    bias=negmr[:jp],
                    scale=rstd_t[:jp],
                )
                nc.tensor.matmul(
                    psd[:],
                    ones[:],
                    sc_sb[:, jt, :],
                    start=(jt == 0),
                    stop=(jt == KT - 1),
                )
                for kt in range(4):
                    nc.tensor.matmul(
                        pp_w1[kt][:, :],
                        wv_sb[h][:, jt, kt * 128 : (kt + 1) * 128],
                        sc_sb[:, jt, :],
                        start=(jt == 0),
                        stop=(jt == KT - 1),
                    )
            r4 = srp.tile([128, C], F32, tag="rd", name=f"r4{h}")
            nc.vector.reciprocal(out=r4[:], in_=psd[:])
            nc.scalar.mul(out=r4[:], in_=r4[:], mul=0.25)

            stage = stgp.tile([128, KT, C], BF16, tag="stg", name=f"stage{h}")

            def pv_out(kt, pp):
                kp = _jp(kt)
                nc.vector.tensor_mul(
                    out=stage[:kp, kt, :], in0=pp[:kp, :], in1=r4[:kp, :]
                )
                if kp < 128:
                    nc.vector.tensor_copy(
                        out=stage[64:128, kt, :], in_=zsrc[64:128, :C]
                    )

            # Wave 2a (kt 4,5) streams on the idle psa banks behind wave 1;
            # wave 2b (kt 6,7) reuses freed pw banks.
            pp_w2a = [
                ps.tile([128, C], F32, tag="psa", bufs=2, name=f"pp{h}w2a_{kt}")
                for kt in range(4, 6)
            ]
            for jt in range(KT):
                for kt in range(4, 6):
                    nc.tensor.matmul(
                        pp_w2a[kt - 4][:, :],
                        wv_sb[h][:, jt, kt * 128 : (kt + 1) * 128],
                        sc_sb[:, jt, :],
                        start=(jt == 0),
                        stop=(jt == KT - 1),
                    )
            for kt in range(4):
                pv_out(kt, pp_w1[kt])
            pp_w2b = [
                ps.tile([128, C], F32, tag="pw", bufs=4, name=f"pp{h}w2b_{kt}")
                for kt in range(6, KT)
            ]
            for jt in range(KT):
                for kt in range(6, KT):
                    kp = _jp(kt)
                    nc.tensor.matmul(
                        pp_w2b[kt - 6][:kp, :],
                        wv_sb[h][:, jt, kt * 128 : kt * 128 + kp],
                        sc_sb[:, jt, :],
                        start=(jt == 0),
                        stop=(jt == KT - 1),
                    )
            for kt in range(4, 6):
                pv_out(kt, pp_w2a[kt - 4])
            for kt in range(6, KT):
                pv_out(kt, pp_w2b[kt - 6])

            if DEBUG and h == 0:
                nc.sync.dma_start(out=dbg_pr.ap()[:], in_=sc_sb[:])
                nc.sync.dma_start(out=dbg_sg.ap()[:], in_=stage[:])
            # per-head pair ReduceScatter of the staged Pv partial
            rs_in = dramp.tile([KVP, 512], BF16, name=f"rs_in{h}")
            for kt in range(KT):
                nc.gpsimd.dma_start(
                    rs_in[kt * 128 : (kt + 1) * 128, :], stage[:, kt, :]
                )
            ro = dramp.tile([512, 512], BF16, name=f"rs_out{h}")
            nc.gpsimd.collective_compute(
                "ReduceScatter",
                mybir.AluOpType.add,
                replica_groups=PAIRS,
                ins=[rs_in[:].opt()],
                outs=[ro[:].opt()],
            )
            rs_out.append(ro)

        if DEBUG:
            nc.sync.dma_start(out=dbg_g.ap()[:], in_=g_sb[:])
        emit_A(0)
        emit_scoresT(0)
        emit_A(1)
        emit_softmax_pv(0)
        emit_scoresT(1)
        emit_softmax_pv(1)
        ph2_pool.__exit__(None, None, None)

        # ---- phase 3: Z = Pbar[own].T-rows @ Wo.T; y_partial = eaL[own] @ Z -
        p3_pool = tc.tile_pool(name="p3ps", bufs=1, space="PSUM")
        ps = p3_pool.__enter__()
        pz = [
            ps.tile([128, C], F32, tag=f"pz{t}", name=f"pz{t}") for t in range(OWN)
        ]
        # Z accumulates the two RS shards in PSUM; the h0 pass runs while the
        # h1 ReduceScatter is still in flight.
        for hh in range(2):
            rst = srp.tile([128, OWN, 512], BF16, tag="sr", name=f"rst{hh}")
            for t in range(OWN):
                nc.sync.dma_start(
                    out=rst[:, t, :],
                    in_=rs_out[hh][t * 128 : (t + 1) * 128, :],
                )
            if DEBUG and hh == 0:
                nc.sync.dma_start(out=dbg_rs.ap()[:], in_=rst[:])
            if DEBUG and hh == 1:
                nc.sync.dma_start(out=dbg_rs1.ap()[:], in_=rst[:])
            pbt = bigp.tile([128, CT, 512], BF16, tag="big", name=f"pbt{hh}")
            for t in range(OWN):
                for dc in range(CT):
                    ptc = ps.tile(
                        [128, 128], BF16, tag="ptc", bufs=2, name=f"zt{hh}{t}{dc}"
                    )
                    nc.tensor.transpose(
                        ptc[:], rst[:, t, dc * 128 : (dc + 1) * 128], ident[:]
                    )
                    dst = pbt[:, dc, t * 128 : (t + 1) * 128]
                    if (t + dc) % 2 == 0:
                        nc.vector.tensor_copy(out=dst, in_=ptc[:])
                    else:
                        nc.scalar.copy(out=dst, in_=ptc[:])
            if DEBUG:
                nc.sync.dma_start(out=dbg_pbt.ap()[hh], in_=pbt[:])
            for t in range(OWN):
                for dc in range(CT):
                    nc.tensor.matmul(
                        pz[t][:],
                        pbt[:, dc, t * 128 : (t + 1) * 128],
                        wot_sb[:, dc, :],
                        start=(hh == 0 and dc == 0),
                        stop=(hh == 1 and dc == CT - 1),
                    )
        z_sb = gp.tile([128, OWN, C], BF16, tag="z", name="z_sb")
        for t in range(OWN):
            if t % 2 == 0:
                nc.vector.tensor_copy(out=z_sb[:, t, :], in_=pz[t][:])
            else:
                nc.scalar.copy(out=z_sb[:, t, :], in_=pz[t][:])

        if DEBUG:
            nc.sync.dma_start(out=dbg_z.ap()[:], in_=z_sb[:])
            nc.sync.dma_start(out=dbg_eat.ap()[:], in_=eat_sb[:, :, 0:512])
        for st in range(ST):
            po = ps.tile([128, C], F32, tag="po", bufs=2, name=f"po{st}")
            for t in range(OWN):
                nc.tensor.matmul(
                    po[:],
                    eat_sb[:, t, st * 128 : (st + 1) * 128],
                    z_sb[:, t, :],
                    start=(t == 0),
                    stop=(t == OWN - 1),
                )
            ot = outp.tile([128, C], BF16, tag="out", name=f"ot{st}")
            if st % 2 == 0:
                nc.scalar.copy(out=ot[:], in_=po[:])
            else:
                nc.vector.tensor_copy(out=ot[:], in_=po[:])
            nc.sync.dma_start(out=y_d.ap()[st * 128 : (st + 1) * 128, :], in_=ot[:])
        p3_pool.__exit__(None, None, None)

    nc.compile()
    return nc


_NC = None


def _get_nc():
    global _NC
    if _NC is None:
        _NC = _build_program()
    return _NC


def _in_maps(emb, emb_all, Wq, Wk, Wv, Wo):
    import ml_dtypes

    def bf(a):
        return np.ascontiguousarray(np.asarray(a, dtype=np.float32)).astype(
            ml_dtypes.bfloat16
        )

    ea32 = np.asarray(emb_all, dtype=np.float32)
    eaP = np.concatenate(
        [ea32, np.zeros((B, S, KVP - KV), np.float32)], axis=2
    )
    wot = np.ascontiguousarray(np.asarray(Wo, dtype=np.float32).T)
    maps = []
    for core in range(8):
        b, g = divmod(core, 2)
        h0 = 2 * g
        if g == 0:
            eaL = eaP[b]
        else:
            eaL = np.concatenate([eaP[b][:, 512:], eaP[b][:, :512]], axis=1)
        maps.append(
            {
                "emb": bf(emb[b]),
                "ea": bf(eaL),
                "wqt": bf(np.asarray(Wq[h0 : h0 + 2]).transpose(0, 2, 1)),
                "wkt": bf(np.asarray(Wk[h0 : h0 + 2]).transpose(0, 2, 1)),
                "wv": bf(Wv[h0 : h0 + 2]),
                "wot": bf(wot),
            }
        )
    return maps


def run(emb, emb_all, Wq, Wk, Wv, Wo, trace=False):
    nc = _get_nc()
    res = run_bass_kernel_spmd(
        nc, _in_maps(emb, emb_all, Wq, Wk, Wv, Wo), list(range(8)), trace=trace
    )
    out = np.empty((B, S, C), dtype=np.float32)
    for b in range(B):
        out[b] = res.results[2 * b]["y"].astype(np.float32) + res.results[
            2 * b + 1
        ]["y"].astype(np.float32)
    return out, res


def kernel(emb, emb_all, Wq, Wk, Wv, Wo):
    out, _ = run(emb, emb_all, Wq, Wk, Wv, Wo, trace=False)
    return out


# revision 4
# speedup vs baseline: 1.0204x; 1.0204x over previous
"""Trainium2 Bass kernel for nn_Attention_1013612281902 (v2: bf16 + pair collectives).

Math (per batch b, head h), restructured to avoid Q/K/V materialization:
    G = emb.T @ emb_all                     [C,KV]  (shared across heads)
    scores_h = Wq[h] @ G @ Wk[h].T          [C,KV]  (unscaled; instance-norm is
                                                     scale-invariant, eps adjusted)
    probs_h  = softmax(instnorm(scores_h))
    Pv_h     = probs_h @ Wv[h]              [C,KV]
    y        = emb_all @ (mean_h Pv_h).T @ Wo.T     [S,C]

Sharding: 8 cores = (4 batches) x (2 pair-members). Within a batch pair:
  - member g owns 4 of the 8 KV k-tiles and computes G only over those columns;
    a pair AllGather assembles the full G on both cores in canonical order.
  - member g computes heads {2g, 2g+1}; per-head Pv partials are pair
    ReduceScattered (sum over members, shard over j2-tiles) so each member
    gets the fully-summed Pbar rows for its own k-tiles.
  - member g computes the partial y = emb_all[:, own] @ Z[own] over ALL rows;
    the host adds the two partials per batch (y is linear in Z's row blocks).
The per-core "own tiles first" view is produced host-side by permuting
emb_all's column blocks per core (eaL), so one fixed SPMD program serves both
pair members; collective rank order restores canonical tile order.

Everything on the PE runs bf16 (f32 PSUM accumulation, f32 stats/softmax
scalars); measured end-to-end error ~6e-3 vs the 2e-2 gate.
"""

import sys

DEBUG = False

if "/opt/trn_rl_repo" not in sys.path:
    sys.path.insert(0, "/opt/trn_rl_repo")

from contextlib import ExitStack

import numpy as np

import concourse.bacc as bacc
import concourse.mybir as mybir
import concourse.tile as tile
from concourse.bass_utils import run_bass_kernel_spmd
from concourse.masks import make_identity
from concourse.tile_rust import add_dep_helper

B, S, C, KV, H = 4, 4096, 512, 960, 4
KVP = 1024              # padded KV
EPS = 1e-5
F32 = mybir.dt.float32
BF16 = mybir.dt.bfloat16

ST = S // 128           # 32 s-tiles
CT = C // 128           # 4 c-tiles
KT = KVP // 128         # 8 k-tiles (padded; tile 7 holds 64 real rows)
OWN = 4                 # owned k-tiles per pair member

PAIRS = [[0, 1], [2, 3], [4, 5], [6, 7]]


def _jp(t):
    """valid j rows in tile t of the unpadded 960 axis"""
    return min(128, KV - t * 128)


def _build_program():
    nc = bacc.Bacc("TRN2", target_bir_lowering=False, debug=False, num_devices=8)

    emb_d = nc.dram_tensor("emb", [S, C], BF16, kind="ExternalInput")
    ea_d = nc.dram_tensor("ea", [S, KVP], BF16, kind="ExternalInput")
    wqt_d = nc.dram_tensor("wqt", [2, C, C], BF16, kind="ExternalInput")
    wkt_d = nc.dram_tensor("wkt", [2, KV, KV], BF16, kind="ExternalInput")
    wv_d = nc.dram_tensor("wv", [2, KV, KV], BF16, kind="ExternalInput")
    wot_d = nc.dram_tensor("wot", [C, C], BF16, kind="ExternalInput")
    y_d = nc.dram_tensor("y", [S, C], BF16, kind="ExternalOutput")
    if DEBUG:
        dbg_g = nc.dram_tensor("dbg_g", [128, CT, KVP], BF16, kind="ExternalOutput")
        dbg_st = nc.dram_tensor("dbg_st", [128, 64], F32, kind="ExternalOutput")
        dbg_pr = nc.dram_tensor("dbg_pr", [128, KT, C], BF16, kind="ExternalOutput")
        dbg_sg = nc.dram_tensor("dbg_sg", [128, KT, C], BF16, kind="ExternalOutput")
        dbg_rs = nc.dram_tensor("dbg_rs", [128, OWN, C], BF16, kind="ExternalOutput")
        dbg_eat = nc.dram_tensor("dbg_eat", [128, OWN, 512], BF16, kind="ExternalOutput")
        dbg_z = nc.dram_tensor("dbg_z", [128, OWN, C], BF16, kind="ExternalOutput")
        dbg_rs1 = nc.dram_tensor("dbg_rs1", [128, OWN, C], BF16, kind="ExternalOutput")
        dbg_pbt = nc.dram_tensor("dbg_pbt", [2, 128, CT, C], BF16, kind="ExternalOutput")

    with tile.TileContext(nc) as tc, ExitStack() as ectx:
        ec = ectx.enter_context
        const = ec(tc.tile_pool(name="const", bufs=1))
        gp = ec(tc.tile_pool(name="gp", bufs=1))
        eatp = ec(tc.tile_pool(name="eatp", bufs=1))
        wqp = ec(tc.tile_pool(name="wqp", bufs=1))
        wkp = ec(tc.tile_pool(name="wkp", bufs=1))
        wvp = ec(tc.tile_pool(name="wvp", bufs=1))
        wop = ec(tc.tile_pool(name="wop", bufs=1))
        embp = ec(tc.tile_pool(name="embp", bufs=6))
        eap = ec(tc.tile_pool(name="eap", bufs=13))
        bigp = ec(tc.tile_pool(name="bigp", bufs=1))   # A tiles (2 heads serialized)
        scp = ec(tc.tile_pool(name="scp", bufs=1))     # scoresT -> probsT
        stgp = ec(tc.tile_pool(name="stgp", bufs=2))   # Pv staging per head
        outp = ec(tc.tile_pool(name="outp", bufs=4))
        srp = ec(tc.tile_pool(name="srp", bufs=2))     # [128,512] scratch
        stp = ec(tc.tile_pool(name="stp", bufs=4))     # small stats tiles
        dramp = ec(tc.tile_pool(name="dramp", bufs=1, space="DRAM"))

        identf = const.tile([128, 128], F32)
        make_identity(nc, identf[:])
        ident = const.tile([128, 128], BF16)
        nc.vector.tensor_copy(out=ident[:], in_=identf[:])
        onesf = const.tile([128, 128], F32)
        nc.vector.memset(onesf[:], 1.0)
        ones = const.tile([128, 128], BF16)
        nc.vector.tensor_copy(out=ones[:], in_=onesf[:])
        # scores are left unscaled (instance-norm is scale-invariant), so the
        # reference's eps applies to var/KV: use KV*eps against unscaled var.
        eps_t = const.tile([128, 1], F32)
        nc.vector.memset(eps_t[:], EPS * KV)
        zsrc = const.tile([128, KV], BF16)
        nc.vector.memset(zsrc[:], 0.0)
        warm = const.tile([128, 1], F32)
        nc.vector.memset(warm[:], 1.0)

        def prewarm(func, nm):
            wsink = stp.tile([128, 1], F32, tag="wsink", name=nm)
            nc.scalar.activation(out=wsink[:], in_=warm[:], func=func)

        # ---- phase 1: G_own = emb.T @ eaL[:, own]  +  eaT transposes --------
        eat_sb = eatp.tile([128, OWN, S], BF16)  # eaL[:, own].T resident
        g_sb = gp.tile([128, CT, KVP], BF16)

        p1_pool = tc.tile_pool(name="p1ps", bufs=1, space="PSUM")
        ps = p1_pool.__enter__()
        g_ps = [
            ps.tile([128, 512], F32, tag=f"g{i}", name=f"g_ps{i}") for i in range(CT)
        ]
        at_tiles = {}

        def emit_tr(st):
            at = at_tiles[st]
            for t in range(OWN):
                ptc = ps.tile([128, 128], BF16, tag="ptc", bufs=3, name=f"p1t{st}{t}")
                nc.tensor.transpose(
                    ptc[:], at[:, t * 128 : (t + 1) * 128], ident[:]
                )
                dst = eat_sb[:, t, st * 128 : (st + 1) * 128]
                if t % 2 == 0:
                    nc.vector.tensor_copy(out=dst, in_=ptc[:])
                else:
                    nc.scalar.copy(out=dst, in_=ptc[:])

        DEFER = 12
        for st in range(ST):
            et = embp.tile([128, C], BF16, tag="emb", name=f"et{st}")
            nc.sync.dma_start(out=et[:], in_=emb_d.ap()[st * 128 : (st + 1) * 128, :])
            at = eap.tile([128, KVP], BF16, tag="ea", name=f"at{st}")
            nc.sync.dma_start(out=at[:], in_=ea_d.ap()[st * 128 : (st + 1) * 128, :])
            at_tiles[st] = at
            for ct in range(CT):
                nc.tensor.matmul(
                    g_ps[ct][:],
                    et[:, ct * 128 : (ct + 1) * 128],
                    at[:, 0:512],
                    start=(st == 0),
                    stop=(st == ST - 1),
                )
            if st < ST - DEFER:
                emit_tr(st)

        # ---- weights (consumption order; stream during AG window) ----------
        wqt_sb = []
        wkt_sb = []
        wv_sb = []
        for h in range(2):
            wq_t = wqp.tile([128, CT, C], BF16, tag="wq", name=f"wq{h}")
            nc.sync.dma_start(
                out=wq_t[:],
                in_=wqt_d.ap()[h].rearrange("(t p) d -> p t d", p=128),
            )
            wqt_sb.append(wq_t)
            wk_t = wkp.tile([128, KT, KV], BF16, tag="wk", name=f"wk{h}")
            wv_t = wvp.tile([128, KT, KV], BF16, tag="wv", name=f"wv{h}")
            for kt in range(KT):
                kp = _jp(kt)
                nc.sync.dma_start(
                    out=wk_t[:kp, kt, :],
                    in_=wkt_d.ap()[h, kt * 128 : kt * 128 + kp, :],
                )
            for kt in range(KT):
                kp = _jp(kt)
                nc.sync.dma_start(
                    out=wv_t[:kp, kt, :],
                    in_=wv_d.ap()[h, kt * 128 : kt * 128 + kp, :],
                )
            nc.vector.tensor_copy(out=wk_t[64:128, KT - 1, :], in_=zsrc[64:128, :])
            nc.vector.tensor_copy(out=wv_t[64:128, KT - 1, :], in_=zsrc[64:128, :])
            wkt_sb.append(wk_t)
            wv_sb.append(wv_t)
        wot_sb = wop.tile([128, CT, C], BF16)
        nc.sync.dma_start(
            out=wot_sb[:], in_=wot_d.ap().rearrange("(t p) d -> p t d", p=128)
        )

        # ---- G AllGather over the pair --------------------------------------
        gtmp = srp.tile([128, CT, 512], BF16, tag="sr", name="gtmp")
        for ct in range(CT):
            if ct % 2 == 0:
                nc.vector.tensor_copy(out=gtmp[:, ct, :], in_=g_ps[ct][:])
            else:
                nc.scalar.copy(out=gtmp[:, ct, :], in_=g_ps[ct][:])
        g_bin = dramp.tile([512, 512], BF16, name="g_bin")
        for ct in range(CT):
            nc.gpsimd.dma_start(
                g_bin[ct * 128 : (ct + 1) * 128, :], gtmp[:, ct, :]
            )
        g_bout = dramp.tile([2, 512, 512], BF16, name="g_bout")
        nc.gpsimd.collective_compute(
            "AllGather",
            mybir.AluOpType.bypass,
            replica_groups=PAIRS,
            ins=[g_bin[:].opt()],
            outs=[g_bout[:].opt()],
        )
        # Deferred transposes + weight-gated junk matmuls fill the AG window
        # (each junk matmul reads a weight chunk, pacing with the DMA stream
        # so HAM doesn't cool the PE during the wait).
        for st in range(ST - DEFER, ST):
            emit_tr(st)
        pwm = ps.tile([128, 128], F32, tag="ptc", bufs=3, name="pwm")
        junk_srcs = [(wqt_sb[0], 0), (wqt_sb[0], 2)] + [
            (wkt_sb[0], kt) for kt in range(KT)
        ] + [(wv_sb[0], kt) for kt in range(0, KT, 2)]
        for i, (wsrc, sub) in enumerate(junk_srcs):
            nc.tensor.matmul(
                pwm[:],
                wsrc[:, sub, 0:128],
                ones[:],
                start=True,
                stop=True,
            )
        for blk in range(2):
            for ct in range(CT):
                nc.sync.dma_start(
                    out=g_sb[:, ct, blk * 512 : (blk + 1) * 512],
                    in_=g_bout[blk, ct * 128 : (ct + 1) * 128, :],
                )
        p1_pool.__exit__(None, None, None)

        # ---- phase 2: per-head scores -> instancenorm -> softmax -> Pv ------
        ph2_pool = tc.tile_pool(name="ph2ps", bufs=1, space="PSUM")
        ps = ph2_pool.__enter__()
        hs = [{}, {}]
        rs_out = []

        def emit_A(h):
            d = hs[h]
            d["a_sb"] = a_sb = bigp.tile(
                [128, KT, C], BF16, tag="big", name=f"a_sb{h}"
            )
            for kt in range(KT):
                pa = ps.tile([128, C], F32, tag="psa", bufs=2, name=f"pa{h}{kt}")
                for ct in range(CT):
                    nc.tensor.matmul(
                        pa[:],
                        g_sb[:, ct, kt * 128 : (kt + 1) * 128],
                        wqt_sb[h][:, ct, :],
                        start=(ct == 0),
                        stop=(ct == CT - 1),
                    )
                nc.vector.tensor_copy(out=a_sb[:, kt, :], in_=pa[:])

        def emit_scoresT(h):
            # scoresT[j, d] = sum_k WkT[k,j] A[k,d]; per-jt stats partials run
            # inline right behind each group.
            d = hs[h]
            a_sb = d["a_sb"]
            d["sc_sb"] = sc_sb = scp.tile(
                [128, KT, C], BF16, tag="sc", name=f"sc_sb{h}"
            )
            d["p_sb"] = p_sb = stp.tile([128, 16], F32, tag="p16", name=f"p_sb{h}")
            nc.vector.memset(p_sb[:], 0.0)
            prev_stop = None
            for jt in range(KT):
                jp = _jp(jt)
                pss = ps.tile([128, C], F32, tag="pw", bufs=4, name=f"pss{h}{jt}")
                for kt in range(KT):
                    mm = nc.tensor.matmul(
                        pss[:jp, :],
                        wkt_sb[h][:, kt, jt * 128 : jt * 128 + jp],
                        a_sb[:, kt, :],
                        start=(kt == 0),
                        stop=(kt == KT - 1),
                    )
                    # Keep the PE stream jt-group-major so stats can chase.
                    if kt == 0 and prev_stop is not None:
                        add_dep_helper(
                            mm.ins, prev_stop.ins, sync=False, reason="jt order"
                        )
                    if kt == KT - 1:
                        prev_stop = mm
                nc.scalar.copy(out=sc_sb[:jp, jt, :], in_=pss[:jp, :])
                nc.vector.reduce_sum(
                    out=p_sb[:jp, jt : jt + 1],
                    in_=pss[:jp, :],
                    axis=mybir.AxisListType.X,
                )
                nc.scalar.activation(
                    out=pss[:jp, :],
                    in_=pss[:jp, :],
                    func=mybir.ActivationFunctionType.Square,
                    accum_out=p_sb[:jp, 8 + jt : 9 + jt],
                )
            nc.vector.tensor_copy(out=sc_sb[64:128, KT - 1, :], in_=zsrc[64:128, :C])

        def emit_softmax_pv(h):
            d = hs[h]
            sc_sb = d["sc_sb"]
            p_sb = d["p_sb"]
            # cross-partition reduce + broadcast of the plane stats.
            p_r = stp.tile([128, 16], BF16, tag="p16r", name=f"p_r{h}")
            nc.vector.tensor_copy(out=p_r[:], in_=p_sb[:])
            pst = ps.tile([128, 16], F32, tag="one", bufs=2, name=f"pst{h}")
            nc.tensor.matmul(pst[:], ones[:], p_r[:], start=True, stop=True)
            n_inv = 1.0 / float(C * KV)
            sq2 = stp.tile([128, 2], F32, tag="sq2", name=f"sq2{h}")
            nc.vector.reduce_sum(
                out=sq2[:],
                in_=pst[:].rearrange("p (a b) -> p a b", a=2),
                axis=mybir.AxisListType.X,
            )
            # mean_neg = -sum/N; em2 = sumsq/N
            mean_neg = stp.tile([128, 1], F32, tag="mean", name=f"mean{h}")
            nc.vector.tensor_scalar(
                out=mean_neg[:], in0=sq2[:, 0:1], scalar1=-n_inv, scalar2=None,
                op0=mybir.AluOpType.mult,
            )
            em2 = stp.tile([128, 1], F32, tag="em2", name=f"em2{h}")
            nc.vector.tensor_scalar(
                out=em2[:], in0=sq2[:, 1:2], scalar1=n_inv, scalar2=None,
                op0=mybir.AluOpType.mult,
            )
            m2 = stp.tile([128, 1], F32, tag="m2", name=f"m2{h}")
            nc.vector.tensor_mul(out=m2[:], in0=mean_neg[:], in1=mean_neg[:])
            var_t = stp.tile([128, 1], F32, tag="var", name=f"var{h}")
            nc.vector.tensor_sub(out=var_t[:], in0=em2[:], in1=m2[:])
            std_t = stp.tile([128, 1], F32, tag="std", name=f"std{h}")
            nc.scalar.activation(
                out=std_t[:],
                in_=var_t[:],
                func=mybir.ActivationFunctionType.Sqrt,
                bias=eps_t[:],
            )
            # Swap the ACT table back to Exp while the DVE finishes the chain.
            prewarm(mybir.ActivationFunctionType.Exp, f"wex{h}")
            rstd_t = stp.tile([128, 1], F32, tag="rstd", name=f"rstd{h}")
            nc.vector.reciprocal(out=rstd_t[:], in_=std_t[:])
            negmr = stp.tile([128, 1], F32, tag="negmr", name=f"negmr{h}")
            nc.vector.tensor_mul(out=negmr[:], in0=mean_neg[:], in1=rstd_t[:])
            if DEBUG and h == 0:
                dstt = stp.tile([128, 64], F32, tag="dstt", name="dstt")
                nc.vector.memset(dstt[:], 0.0)
                nc.vector.tensor_copy(out=dstt[:, 0:16], in_=p_sb[:])
                nc.vector.tensor_copy(out=dstt[:, 16:17], in_=mean_neg[:])
                nc.vector.tensor_copy(out=dstt[:, 17:18], in_=var_t[:])
                nc.vector.tensor_copy(out=dstt[:, 18:19], in_=rstd_t[:])
                nc.vector.tensor_copy(out=dstt[:, 19:20], in_=negmr[:])
                nc.vector.tensor_copy(out=dstt[:, 20:36], in_=pst[:])
                nc.sync.dma_start(out=dbg_st.ap()[:], in_=dstt[:])

            # softmax + Pv fused: Pv matmuls consume raw exp tiles as they
            # are produced; 1/denom and the 0.25 head-mean factor fold into
            # the staging copy-out.
            psd = ps.tile([128, C], F32, tag="one", bufs=2, name=f"psd{h}")
            pp_w1 = [
                ps.tile([128, C], F32, tag="pw", bufs=4, name=f"pp{h}w1_{kt}")
                for kt in range(4)
            ]
            for jt in range(KT):
                jp = _jp(jt)
                nc.scalar.activation(
                    out=sc_sb[:jp, jt, :],
                    in_=sc_sb[:jp, jt, :],
                    func=mybir.ActivationFunctionType.Exp,
                    bias=negmr[:jp],
                    scale=rstd_t[:jp],
                )
                nc.tensor.matmul(
                    psd[:],
                    ones[:],
                    sc_sb[:, jt, :],
                    start=(jt == 0),
                    stop=(jt == KT - 1),
                )
                for kt in range(4):
                    nc.tensor.matmul(
                        pp_w1[kt][:, :],
                        wv_sb[h][:, jt, kt * 128 : (kt + 1) * 128],
                        sc_sb[:, jt, :],
                        start=(jt == 0),
                        stop=(jt == KT - 1),
                    )
            r4 = srp.tile([128, C], F32, tag="rd", name=f"r4{h}")
            nc.vector.reciprocal(out=r4[:], in_=psd[:])
            nc.scalar.mul(out=r4[:], in_=r4[:], mul=0.25)

            stage = stgp.tile([128, KT, C], BF16, tag="stg", name=f"stage{h}")

            def pv_out(kt, pp):
                kp = _jp(kt)
                nc.vector.tensor_mul(
                    out=stage[:kp, kt, :], in0=pp[:kp, :], in1=r4[:kp, :]
                )
                if kp < 128:
                    nc.vector.tensor_copy(
                        out=stage[64:128, kt, :], in_=zsrc[64:128, :C]
                    )

            # Wave 2a (kt 4,5) streams on the idle psa banks behind wave 1;
            # wave 2b (kt 6,7) reuses freed pw banks.
            pp_w2a = [
                ps.tile([128, C], F32, tag="psa", bufs=2, name=f"pp{h}w2a_{kt}")
                for kt in range(4, 6)
            ]
            for jt in range(KT):
                for kt in range(4, 6):
                    nc.tensor.matmul(
                        pp_w2a[kt - 4][:, :],
                        wv_sb[h][:, jt, kt * 128 : (kt + 1) * 128],
                        sc_sb[:, jt, :],
                        start=(jt == 0),
                        stop=(jt == KT - 1),
                    )
            for kt in range(4):
                pv_out(kt, pp_w1[kt])
            pp_w2b = [
                ps.tile([128, C], F32, tag="pw", bufs=4, name=f"pp{h}w2b_{kt}")
                for kt in range(6, KT)
            ]
            for jt in range(KT):
                for kt in range(6, KT):
                    kp = _jp(kt)
                    nc.tensor.matmul(
                        pp_w2b[kt - 6][:kp, :],
                        wv_sb[h][:, jt, kt * 128 : kt * 128 + kp],
                        sc_sb[:, jt, :],
                        start=(jt == 0),
                        stop=(jt == KT - 1),
                    )
            for kt in range(4, 6):
                pv_out(kt, pp_w2a[kt - 4])
            for kt in range(6, KT):
                pv_out(kt, pp_w2b[kt - 6])

            if DEBUG and h == 0:
                nc.sync.dma_start(out=dbg_pr.ap()[:], in_=sc_sb[:])
                nc.sync.dma_start(out=dbg_sg.ap()[:], in_=stage[:])
            # per-head pair ReduceScatter of the staged Pv partial
            rs_in = dramp.tile([KVP, 512], BF16, name=f"rs_in{h}")
            for kt in range(KT):
                nc.gpsimd.dma_start(
                    rs_in[kt * 128 : (kt + 1) * 128, :], stage[:, kt, :]
                )
            ro = dramp.tile([512, 512], BF16, name=f"rs_out{h}")
            nc.gpsimd.collective_compute(
                "ReduceScatter",
                mybir.AluOpType.add,
                replica_groups=PAIRS,
                ins=[rs_in[:].opt()],
                outs=[ro[:].opt()],
            )
            rs_out.append(ro)

        if DEBUG:
            nc.sync.dma_start(out=dbg_g.ap()[:], in_=g_sb[:])
        def emit_stats_bridge(h, n):
            # Junk matmuls with no deps keep the PE busy across the serial
            # stats chain between scoresT(h) and softmax_pv(h).
            pj = ps.tile([128, 128], F32, tag="one", bufs=2, name=f"pj{h}")
            for i in range(n):
                nc.tensor.matmul(pj[:], ones[:], ident[:], start=True, stop=True)

        emit_A(0)
        emit_scoresT(0)
        emit_A(1)
        emit_stats_bridge(0, 24)
        emit_softmax_pv(0)
        emit_scoresT(1)
        emit_stats_bridge(1, 48)
        emit_softmax_pv(1)
        ph2_pool.__exit__(None, None, None)

        # ---- phase 3: Z = Pbar[own].T-rows @ Wo.T; y_partial = eaL[own] @ Z -
        p3_pool = tc.tile_pool(name="p3ps", bufs=1, space="PSUM")
        ps = p3_pool.__enter__()
        pz = [
            ps.tile([128, C], F32, tag=f"pz{t}", name=f"pz{t}") for t in range(OWN)
        ]
        # Z accumulates the two RS shards in PSUM; the h0 pass runs while the
        # h1 ReduceScatter is still in flight.
        for hh in range(2):
            rst = srp.tile([128, OWN, 512], BF16, tag="sr", name=f"rst{hh}")
            for t in range(OWN):
                nc.sync.dma_start(
                    out=rst[:, t, :],
                    in_=rs_out[hh][t * 128 : (t + 1) * 128, :],
                )
            if DEBUG and hh == 0:
                nc.sync.dma_start(out=dbg_rs.ap()[:], in_=rst[:])
            if DEBUG and hh == 1:
                nc.sync.dma_start(out=dbg_rs1.ap()[:], in_=rst[:])
            pbt = bigp.tile([128, CT, 512], BF16, tag="big", name=f"pbt{hh}")
            for t in range(OWN):
                for dc in range(CT):
                    ptc = ps.tile(
                        [128, 128], BF16, tag="ptc", bufs=2, name=f"zt{hh}{t}{dc}"
                    )
                    nc.tensor.transpose(
                        ptc[:], rst[:, t, dc * 128 : (dc + 1) * 128], ident[:]
                    )
                    dst = pbt[:, dc, t * 128 : (t + 1) * 128]
                    if (t + dc) % 2 == 0:
                        nc.vector.tensor_copy(out=dst, in_=ptc[:])
                    else:
                        nc.scalar.copy(out=dst, in_=ptc[:])
            if DEBUG:
                nc.sync.dma_start(out=dbg_pbt.ap()[hh], in_=pbt[:])
            for t in range(OWN):
                for dc in range(CT):
                    nc.tensor.matmul(
                        pz[t][:],
                        pbt[:, dc, t * 128 : (t + 1) * 128],
                        wot_sb[:, dc, :],
                        start=(hh == 0 and dc == 0),
                        stop=(hh == 1 and dc == CT - 1),
                    )
            if hh == 0:
                pjt = ps.tile([128, 128], F32, tag="ptc", bufs=2, name="pjt")
                for i in range(40):
                    nc.tensor.matmul(
                        pjt[:], ones[:], ident[:], start=True, stop=True
                    )
        z_sb = gp.tile([128, OWN, C], BF16, tag="z", name="z_sb")
        for t in range(OWN):
            if t % 2 == 0:
                nc.vector.tensor_copy(out=z_sb[:, t, :], in_=pz[t][:])
            else:
                nc.scalar.copy(out=z_sb[:, t, :], in_=pz[t][:])

        if DEBUG:
            nc.sync.dma_start(out=dbg_z.ap()[:], in_=z_sb[:])
            nc.sync.dma_start(out=dbg_eat.ap()[:], in_=eat_sb[:, :, 0:512])
        for st in range(ST):
            po = ps.tile([128, C], F32, tag="po", bufs=2, name=f"po{st}")
            for t in range(OWN):
                nc.tensor.matmul(
                    po[:],
                    eat_sb[:, t, st * 128 : (st + 1) * 128],
                    z_sb[:, t, :],
                    start=(t == 0),
                    stop=(t == OWN - 1),
                )
            ot = outp.tile([128, C], BF16, tag="out", name=f"ot{st}")
            if st % 2 == 0:
                nc.scalar.copy(out=ot[:], in_=po[:])
            else:
                nc.vector.tensor_copy(out=ot[:], in_=po[:])
            nc.sync.dma_start(out=y_d.ap()[st * 128 : (st + 1) * 128, :], in_=ot[:])
        p3_pool.__exit__(None, None, None)

    nc.compile()
    return nc


_NC = None


def _get_nc():
    global _NC
    if _NC is None:
        _NC = _build_program()
    return _NC


def _in_maps(emb, emb_all, Wq, Wk, Wv, Wo):
    import ml_dtypes

    def bf(a):
        return np.ascontiguousarray(np.asarray(a, dtype=np.float32)).astype(
            ml_dtypes.bfloat16
        )

    ea32 = np.asarray(emb_all, dtype=np.float32)
    eaP = np.concatenate(
        [ea32, np.zeros((B, S, KVP - KV), np.float32)], axis=2
    )
    wot = np.ascontiguousarray(np.asarray(Wo, dtype=np.float32).T)
    maps = []
    for core in range(8):
        b, g = divmod(core, 2)
        h0 = 2 * g
        if g == 0:
            eaL = eaP[b]
        else:
            eaL = np.concatenate([eaP[b][:, 512:], eaP[b][:, :512]], axis=1)
        maps.append(
            {
                "emb": bf(emb[b]),
                "ea": bf(eaL),
                "wqt": bf(np.asarray(Wq[h0 : h0 + 2]).transpose(0, 2, 1)),
                "wkt": bf(np.asarray(Wk[h0 : h0 + 2]).transpose(0, 2, 1)),
                "wv": bf(Wv[h0 : h0 + 2]),
                "wot": bf(wot),
            }
        )
    return maps


def run(emb, emb_all, Wq, Wk, Wv, Wo, trace=False):
    nc = _get_nc()
    res = run_bass_kernel_spmd(
        nc, _in_maps(emb, emb_all, Wq, Wk, Wv, Wo), list(range(8)), trace=trace
    )
    out = np.empty((B, S, C), dtype=np.float32)
    for b in range(B):
        out[b] = res.results[2 * b]["y"].astype(np.float32) + res.results[
            2 * b + 1
        ]["y"].astype(np.float32)
    return out, res


def kernel(emb, emb_all, Wq, Wk, Wv, Wo):
    out, _ = run(emb, emb_all, Wq, Wk, Wv, Wo, trace=False)
    return out
